# revision 28
# baseline (speedup 1.0000x reference)
"""LSTM (B=512, T=512, D=32, H=64) + sigmoid linear head on 8 NeuronCores.

Data-parallel over batch (64 per core); the T=512 recurrence runs locally
per core. Everything lives in transposed [feature, batch] layout so the
per-step matmul contracts over partitions.

Structure (v2):
  - The x-projection (W_ih @ x + biases) is bulk-matmul'ed per 8-step
    chunk straight into PSUM (K=33 incl. a ones-row for the biases);
    the per-step W_hh @ h matmuls then accumulate onto it (start=False),
    so each step needs only 2 small K=64 matmuls on the critical path.
  - g-gate rows of the weights are pre-scaled by 2 so ONE sigmoid op
    covers i/f/o AND tanh(g) (tanh(g) = 2*sigmoid(2g) - 1).
  - cell state is stored halved (c' = c/2):
        c' = f*c' + i*(sigmoid(2g) - 0.5)
    f*c' runs on GpSimd in parallel with the fused scalar_tensor_tensor
    on DVE; tanh(c) = Tanh(2*c') uses the activation's free input scale.
  - h_t is written directly into the next step's matmul-rhs tile (which
    doubles as the hs output staging buffer).
  - The linear head runs on GpSimd per 64-step chunk: per-partition
    scale by W_out then a partition-sum reduce into one accumulator row;
    one sigmoid + DMA at the very end.
"""

from contextlib import ExitStack

import numpy as np

import concourse.bacc as bacc
import concourse.bass as bass
import concourse.mybir as mybir
import concourse.tile as tile
from concourse.bass_utils import run_bass_kernel_spmd
from concourse.tile import add_dep_helper

B, T, D, H = 512, 512, 32, 64
NCORES = 8
BL = B // NCORES  # 64 batch per core
KX = D + 1  # 33: [x; ones]
TC = 64  # timesteps per sbuf chunk tile
TCP = 8  # timesteps per psum xg chunk
F32 = mybir.dt.float32
BF16 = mybir.dt.float16
AF = mybir.ActivationFunctionType
ALU = mybir.AluOpType


def build_program(t_steps: int = T):
    tcs = min(TC, t_steps)
    tcp = min(TCP, t_steps)
    nch = (t_steps + tcs - 1) // tcs
    nc = bacc.Bacc()
    xa = nc.declare_dram_parameter("xa", [KX, t_steps, BL], BF16, False)
    h0 = nc.declare_dram_parameter("h0", [H, BL], BF16, False)
    c0h = nc.declare_dram_parameter("c0h", [H, BL], F32, False)
    wha = nc.declare_dram_parameter("wha", [H, 2 * H], BF16, False)
    whb = nc.declare_dram_parameter("whb", [H, 2 * H], BF16, False)
    wxa = nc.declare_dram_parameter("wxa", [KX, 2 * H], BF16, False)
    wxb = nc.declare_dram_parameter("wxb", [KX, 2 * H], BF16, False)
    wo = nc.declare_dram_parameter("wo", [H, H], BF16, False)
    bo = nc.declare_dram_parameter("bo", [1, 1], F32, False)
    hs = nc.declare_dram_parameter("hs", [H, t_steps + 1, BL], BF16, True)
    hd = nc.declare_dram_parameter("hd", [1, (t_steps + 1) * BL], F32, True)

    xa_r = xa.rearrange("d t b -> d (t b)")
    hs_r = hs.rearrange("h t b -> h (t b)")

    with tile.TileContext(nc) as tc, ExitStack() as ctx:
        const_pool = ctx.enter_context(tc.tile_pool(name="const", bufs=1))
        h_pool = ctx.enter_context(tc.tile_pool(name="hbuf", bufs=4))
        x_pool = ctx.enter_context(tc.tile_pool(name="xbuf", bufs=3))
        hd_pool = ctx.enter_context(tc.tile_pool(name="hdbuf", bufs=2))
        sig_pool = ctx.enter_context(tc.tile_pool(name="sig", bufs=8))
        small_pool = ctx.enter_context(tc.tile_pool(name="small", bufs=8))
        state_pool = ctx.enter_context(tc.tile_pool(name="state", bufs=8))
        gpsum_pool = ctx.enter_context(
            tc.tile_pool(name="gpsum", bufs=2, space="PSUM")
        )
        hpsum_pool = ctx.enter_context(
            tc.tile_pool(name="hpsum", bufs=2, space="PSUM")
        )

        wha_t = const_pool.tile([H, 2 * H], BF16, tag="wha")
        nc.sync.dma_start(wha_t[:], wha[:])
        whb_t = const_pool.tile([H, 2 * H], BF16, tag="whb")
        nc.sync.dma_start(whb_t[:], whb[:])
        wxa_t = const_pool.tile([KX, 2 * H], BF16, tag="wxa")
        nc.sync.dma_start(wxa_t[:], wxa[:])
        wxb_t = const_pool.tile([KX, 2 * H], BF16, tag="wxb")
        nc.sync.dma_start(wxb_t[:], wxb[:])
        wo_t = const_pool.tile([H, H], BF16, tag="wo")
        nc.sync.dma_start(wo_t[:], wo[:])
        bo_t = const_pool.tile([1, 1], F32, tag="bo")
        nc.sync.dma_start(bo_t[:], bo[:])

        def alloc_hchunk():
            ht_ = h_pool.tile([H, tcs * BL], BF16, tag="hchunk")
            return ht_

        def alloc_xchunk(c):
            t_ = x_pool.tile([KX, tcs * BL], BF16, tag="xchunk")
            nc.sync.dma_start(
                t_[:, :], xa_r[:, c * tcs * BL : (c + 1) * tcs * BL]
            )
            return t_

        h_tiles = [alloc_hchunk()]
        x_tiles = {0: alloc_xchunk(0)}
        nc.sync.dma_start(h_tiles[0][:, 0:BL], h0[:])
        stub = hd_pool.tile([H, BL], BF16, tag="stub")

        c_state = state_pool.tile([128, BL], F32, tag="c")
        nc.sync.dma_start(c_state[H:128, :], c0h[:])

        def emit_xg(tstart):
            # bulk x-projection for steps [tstart, tstart+tcp) into PSUM
            xg_ = gpsum_pool.tile([128, 2, tcp * BL], F32, tag="xg")
            xc, xj = divmod(tstart, tcs)
            xsl = x_tiles[xc][:, xj * BL : xj * BL + tcp * BL]
            b0 = nc.tensor.matmul(
                xg_[:, 0, :], wxa_t[:], xsl, start=True, stop=False
            )
            b1 = nc.tensor.matmul(
                xg_[:, 1, :], wxb_t[:], xsl, start=True, stop=False
            )
            return xg_, [b0.ins, b1.ins]

        xg_cur, _ = emit_xg(0)
        xg_nxt = None
        for t in range(t_steps):
            c, j = divmod(t, tcs)
            cp, jp = divmod(t, tcp)
            cur = h_tiles[c]
            if j == 0 and c + 1 < nch:
                h_tiles.append(alloc_hchunk())
                x_tiles[c + 1] = alloc_xchunk(c + 1)
            if jp == 0 and t > 0:
                xg_cur, xg_nxt = xg_nxt, None
            xg_ps = xg_cur
            if t + 1 < t_steps:
                nxt = h_tiles[(t + 1) // tcs]
                jn = (t + 1) % tcs
                hdst = nxt[:, jn * BL : (jn + 1) * BL]
            else:
                hdst = stub[:, 0:BL]

            rhs_ap = cur[:, j * BL : (j + 1) * BL]
            mm_a_i = nc.tensor.matmul(
                xg_ps[:, 0, jp * BL : (jp + 1) * BL],
                wha_t[:],
                rhs_ap,
                start=False,
                stop=(jp == tcp - 1),
                skip_group_check=True,
            )
            mm_b_i = nc.tensor.matmul(
                xg_ps[:, 1, jp * BL : (jp + 1) * BL],
                whb_t[:],
                rhs_ap,
                start=False,
                stop=(jp == tcp - 1),
                skip_group_check=True,
            )
            # s: cols 0:BL = [sig_i; sig_f], cols BL:2BL = [sig_2g; sig_o]
            s_t = sig_pool.tile([128, 2, BL], F32, tag="s")
            nc.scalar.activation(
                s_t[:], xg_ps[:, :, jp * BL : (jp + 1) * BL], AF.Sigmoid
            )

            r_t = small_pool.tile([128, BL], F32, tag="r")
            nc.vector.tensor_mul(
                r_t[H:128, :], s_t[H:128, 0, :], c_state[H:128, :]
            )  # f*c'
            q_t = small_pool.tile([128, BL], F32, tag="q")
            nc.vector.scalar_tensor_tensor(
                q_t[H:128, :],
                s_t[0:H, 1, :],
                0.5,
                s_t[0:H, 0, :],
                ALU.subtract,
                ALU.mult,
            )  # (sig_2g - 0.5) * i
            c_new = state_pool.tile([128, BL], F32, tag="c")
            nc.vector.tensor_add(c_new[H:128, :], r_t[H:128, :], q_t[H:128, :])
            c_state = c_new

            tc_t = small_pool.tile([128, BL], F32, tag="tc")
            tanh_i = nc.scalar.activation(
                tc_t[H:128, :], c_new[H:128, :], AF.Tanh, scale=2.0
            )
            nc.vector.tensor_mul(hdst, s_t[H:128, 1, :], tc_t[H:128, :])  # o*tanh

            if jp == 0:
                if t + tcp < t_steps:
                    # prefetch next psum-chunk's x-projection; force it
                    # behind this step's gate matmuls so it fills the idle
                    # PE window instead of delaying the critical path
                    xg_nxt, xg_bis = emit_xg(t + tcp)
                    for bi in xg_bis:
                        add_dep_helper(bi, mm_b_i.ins, False, "fill-idle")
                if t > 0:
                    # head for slice [t-tcp, t): one matmul with replicated
                    # W_out stationary (all psum rows identical); sigmoid
                    # only row 0
                    a0 = t - tcp
                    hc0, hj0 = divmod(a0, tcs)
                    hsl = h_tiles[hc0][:, hj0 * BL : (hj0 + tcp) * BL]
                    hp = hpsum_pool.tile([H, tcp * BL], F32, tag="hp")
                    hmm_i = nc.tensor.matmul(
                        hp[:], wo_t[:], hsl, start=True, stop=True
                    )
                    add_dep_helper(hmm_i.ins, mm_b_i.ins, False, "fill-idle")
                    hb = hd_pool.tile([1, tcp * BL], F32, tag="hb")
                    hsig_i = nc.scalar.activation(
                        hb[:], hp[0:1, :], AF.Sigmoid, bias=bo_t[0:1, 0:1]
                    )
                    add_dep_helper(hsig_i.ins, tanh_i.ins, False, "fill-idle")
                    nc.sync.dma_start(
                        hd[0:1, a0 * BL : (a0 + tcp) * BL], hb[:]
                    )
            if j == tcs - 1:
                nc.sync.dma_start(
                    hs_r[:, c * tcs * BL : (c + 1) * tcs * BL], cur[:, :]
                )

        nc.sync.dma_start(hs_r[:, t_steps * BL : (t_steps + 1) * BL], stub[:, :])
        # head for the last tcp slice
        la0 = t_steps - tcp
        lc0, lj0 = divmod(la0, tcs)
        lsl = h_tiles[lc0][:, lj0 * BL : (lj0 + tcp) * BL]
        lhp = hpsum_pool.tile([H, tcp * BL], F32, tag="hp")
        nc.tensor.matmul(lhp[:], wo_t[:], lsl, start=True, stop=True)
        lhb = hd_pool.tile([1, tcp * BL], F32, tag="hb")
        nc.scalar.activation(
            lhb[:], lhp[0:1, :], AF.Sigmoid, bias=bo_t[0:1, 0:1]
        )
        nc.sync.dma_start(hd[0:1, la0 * BL : (la0 + tcp) * BL], lhb[:])
        # head for the final h^{(T)}
        hps = hpsum_pool.tile([H, BL], F32, tag="hps")
        nc.tensor.matmul(hps[:], wo_t[:], stub[:, :], start=True, stop=True)
        hbs = hd_pool.tile([1, BL], F32, tag="hbs")
        nc.scalar.activation(
            hbs[:], hps[0:1, :], AF.Sigmoid, bias=bo_t[0:1, 0:1]
        )
        nc.sync.dma_start(
            hd[0:1, t_steps * BL : (t_steps + 1) * BL], hbs[:]
        )

    nc.finalize()
    return nc


def make_in_maps(x, h_0, c_0, W_ih, W_hh, b_ih, b_hh, W_out, b_out):
    x = np.asarray(x, np.float32)
    t_steps = x.shape[1]
    nch = (t_steps + TC - 1) // TC
    Wh = np.asarray(W_hh, np.float32).copy()  # [4H, H]
    Wx = np.concatenate(
        [
            np.asarray(W_ih, np.float32),
            (np.asarray(b_ih, np.float32) + np.asarray(b_hh, np.float32))[:, None],
        ],
        axis=1,
    ).copy()  # [4H, 33]
    Wh[2 * H : 3 * H, :] *= 2.0  # g rows: sigmoid(2g) trick
    Wx[2 * H : 3 * H, :] *= 2.0
    wha = np.ascontiguousarray(Wh[0 : 2 * H, :].T)  # [64, 128] (i, f)
    whb = np.ascontiguousarray(Wh[2 * H : 4 * H, :].T)  # [64, 128] (2g, o)
    wxa = np.ascontiguousarray(Wx[0 : 2 * H, :].T)  # [33, 128]
    wxb = np.ascontiguousarray(Wx[2 * H : 4 * H, :].T)  # [33, 128]
    bf = np.float16
    wo = np.ascontiguousarray(
        np.tile(np.asarray(W_out, np.float32).T, (1, H))
    ).astype(bf)  # [H, H]: column m = W_out for every m
    bscalar = float(np.asarray(b_out, np.float32).reshape(-1)[0])

    xa_full = np.concatenate(
        [x, np.ones((x.shape[0], t_steps, 1), np.float32)], axis=2
    )  # [B, T, 33]
    xa_full = np.ascontiguousarray(xa_full.transpose(2, 1, 0))  # [33, T, B]
    h0_full = np.ascontiguousarray(np.asarray(h_0, np.float32)[0].T)  # [H, B]
    c0_full = np.ascontiguousarray(np.asarray(c_0, np.float32)[0].T) * 0.5

    in_maps = []
    for k in range(NCORES):
        sl = slice(k * BL, (k + 1) * BL)
        in_maps.append(
            {
                "xa": np.ascontiguousarray(xa_full[:, :, sl]).astype(bf),
                "h0": np.ascontiguousarray(h0_full[:, sl]).astype(bf),
                "c0h": np.ascontiguousarray(c0_full[:, sl]),
                "wha": wha.astype(bf),
                "whb": whb.astype(bf),
                "wxa": wxa.astype(bf),
                "wxb": wxb.astype(bf),
                "wo": wo,
                "bo": np.full((1, 1), bscalar, np.float32),
            }
        )
    return in_maps


def assemble_outputs(results, t_steps: int = T):
    bsz = NCORES * BL
    tcs = min(TC, t_steps)
    nch = (t_steps + tcs - 1) // tcs
    hs_out = np.empty((bsz, t_steps, H), np.float32)
    out = np.empty((bsz, t_steps, 1), np.float32)
    for k in range(NCORES):
        hs_k = np.asarray(results[k]["hs"]).astype(np.float32)  # [H, T+1, BL]
        hs_out[k * BL : (k + 1) * BL] = hs_k[:, 1:, :].transpose(2, 1, 0)
        hd_k = np.asarray(results[k]["hd"])  # [1, (T+1)*BL]
        heads = hd_k.reshape(t_steps + 1, BL)
        out[k * BL : (k + 1) * BL, :, 0] = heads[1 : t_steps + 1, :].T
    return out, hs_out


def kernel(x, h_0, c_0, W_ih, W_hh, b_ih, b_hh, W_out, b_out):
    import time

    in_maps = make_in_maps(x, h_0, c_0, W_ih, W_hh, b_ih, b_hh, W_out, b_out)
    nc = build_program(T)
    last_err = None
    for attempt in range(3):
        try:
            res = run_bass_kernel_spmd(nc, in_maps, list(range(NCORES))).results
            return assemble_outputs(res, T)
        except Exception as e:  # transient NRT device errors: retry
            last_err = e
            time.sleep(10.0)
    raise last_err


if __name__ == "__main__":
    nc = build_program(T)
    print("build ok")


# revision 29
# speedup vs baseline: 1.0000x; 1.0000x over previous
"""LSTM (B=512, T=512, D=32, H=64) + sigmoid linear head on 8 NeuronCores.

Data-parallel over batch (64 per core); the T=512 recurrence runs locally
per core. Everything lives in transposed [feature, batch] layout so the
per-step matmul contracts over partitions.

Structure (v2):
  - The x-projection (W_ih @ x + biases) is bulk-matmul'ed per 8-step
    chunk straight into PSUM (K=33 incl. a ones-row for the biases);
    the per-step W_hh @ h matmuls then accumulate onto it (start=False),
    so each step needs only 2 small K=64 matmuls on the critical path.
  - g-gate rows of the weights are pre-scaled by 2 so ONE sigmoid op
    covers i/f/o AND tanh(g) (tanh(g) = 2*sigmoid(2g) - 1).
  - cell state is stored halved (c' = c/2):
        c' = f*c' + i*(sigmoid(2g) - 0.5)
    f*c' runs on GpSimd in parallel with the fused scalar_tensor_tensor
    on DVE; tanh(c) = Tanh(2*c') uses the activation's free input scale.
  - h_t is written directly into the next step's matmul-rhs tile (which
    doubles as the hs output staging buffer).
  - The linear head runs on GpSimd per 64-step chunk: per-partition
    scale by W_out then a partition-sum reduce into one accumulator row;
    one sigmoid + DMA at the very end.
"""

from contextlib import ExitStack

import numpy as np

import concourse.bacc as bacc
import concourse.bass as bass
import concourse.mybir as mybir
import concourse.tile as tile
from concourse.bass_utils import run_bass_kernel_spmd
from concourse.tile import add_dep_helper

B, T, D, H = 512, 512, 32, 64
NCORES = 8
BL = B // NCORES  # 64 batch per core
KX = D + 1  # 33: [x; ones]
TC = 64  # timesteps per sbuf chunk tile
TCP = 8  # timesteps per psum xg chunk
F32 = mybir.dt.float32
BF16 = mybir.dt.float16
AF = mybir.ActivationFunctionType
ALU = mybir.AluOpType


def build_program(t_steps: int = T):
    tcs = min(TC, t_steps)
    tcp = min(TCP, t_steps)
    nch = (t_steps + tcs - 1) // tcs
    nc = bacc.Bacc()
    xa = nc.declare_dram_parameter("xa", [KX, t_steps, BL], BF16, False)
    h0 = nc.declare_dram_parameter("h0", [H, BL], BF16, False)
    c0h = nc.declare_dram_parameter("c0h", [H, BL], F32, False)
    wha = nc.declare_dram_parameter("wha", [H, 2 * H], BF16, False)
    whb = nc.declare_dram_parameter("whb", [H, 2 * H], BF16, False)
    wxa = nc.declare_dram_parameter("wxa", [KX, 2 * H], BF16, False)
    wxb = nc.declare_dram_parameter("wxb", [KX, 2 * H], BF16, False)
    wo = nc.declare_dram_parameter("wo", [H, H], BF16, False)
    bo = nc.declare_dram_parameter("bo", [1, 1], F32, False)
    hs = nc.declare_dram_parameter("hs", [H, t_steps + 1, BL], BF16, True)
    hd = nc.declare_dram_parameter("hd", [1, (t_steps + 1) * BL], F32, True)

    xa_r = xa.rearrange("d t b -> d (t b)")
    hs_r = hs.rearrange("h t b -> h (t b)")

    with tile.TileContext(nc) as tc, ExitStack() as ctx:
        const_pool = ctx.enter_context(tc.tile_pool(name="const", bufs=1))
        h_pool = ctx.enter_context(tc.tile_pool(name="hbuf", bufs=4))
        x_pool = ctx.enter_context(tc.tile_pool(name="xbuf", bufs=3))
        hd_pool = ctx.enter_context(tc.tile_pool(name="hdbuf", bufs=2))
        sig_pool = ctx.enter_context(tc.tile_pool(name="sig", bufs=8))
        small_pool = ctx.enter_context(tc.tile_pool(name="small", bufs=8))
        state_pool = ctx.enter_context(tc.tile_pool(name="state", bufs=8))
        gpsum_pool = ctx.enter_context(
            tc.tile_pool(name="gpsum", bufs=2, space="PSUM")
        )
        hpsum_pool = ctx.enter_context(
            tc.tile_pool(name="hpsum", bufs=1, space="PSUM")
        )
        warm_pool = ctx.enter_context(
            tc.tile_pool(name="warm", bufs=1, space="PSUM")
        )

        wha_t = const_pool.tile([H, 2 * H], BF16, tag="wha")
        nc.sync.dma_start(wha_t[:], wha[:])
        whb_t = const_pool.tile([H, 2 * H], BF16, tag="whb")
        nc.sync.dma_start(whb_t[:], whb[:])
        wxa_t = const_pool.tile([KX, 2 * H], BF16, tag="wxa")
        nc.sync.dma_start(wxa_t[:], wxa[:])
        wxb_t = const_pool.tile([KX, 2 * H], BF16, tag="wxb")
        nc.sync.dma_start(wxb_t[:], wxb[:])
        wo_t = const_pool.tile([H, H], BF16, tag="wo")
        nc.sync.dma_start(wo_t[:], wo[:])
        bo_t = const_pool.tile([1, 1], F32, tag="bo")
        nc.sync.dma_start(bo_t[:], bo[:])

        def alloc_hchunk():
            ht_ = h_pool.tile([H, tcs * BL], BF16, tag="hchunk")
            return ht_

        def alloc_xchunk(c):
            t_ = x_pool.tile([KX, tcs * BL], BF16, tag="xchunk")
            nc.sync.dma_start(
                t_[:, :], xa_r[:, c * tcs * BL : (c + 1) * tcs * BL]
            )
            return t_

        h_tiles = [alloc_hchunk()]
        x_tiles = {0: alloc_xchunk(0)}
        nc.sync.dma_start(h_tiles[0][:, 0:BL], h0[:])
        stub = hd_pool.tile([H, BL], BF16, tag="stub")

        c_state = state_pool.tile([128, BL], F32, tag="c")
        nc.sync.dma_start(c_state[H:128, :], c0h[:])

        dummy = const_pool.tile([H, 8 * BL], BF16, tag="dummy")
        nc.gpsimd.memset(dummy[:], 0.0)
        warm_ps = warm_pool.tile([128, 8 * BL], F32, tag="warm")

        def emit_xg(tstart):
            # bulk x-projection for steps [tstart, tstart+tcp) into PSUM
            xg_ = gpsum_pool.tile([128, 2, tcp * BL], F32, tag="xg")
            xc, xj = divmod(tstart, tcs)
            xsl = x_tiles[xc][:, xj * BL : xj * BL + tcp * BL]
            b0 = nc.tensor.matmul(
                xg_[:, 0, :], wxa_t[:], xsl, start=True, stop=False
            )
            b1 = nc.tensor.matmul(
                xg_[:, 1, :], wxb_t[:], xsl, start=True, stop=False
            )
            return xg_, [b0.ins, b1.ins]

        xg_cur, _ = emit_xg(0)
        xg_nxt = None
        for t in range(t_steps):
            c, j = divmod(t, tcs)
            cp, jp = divmod(t, tcp)
            cur = h_tiles[c]
            if j == 0 and c + 1 < nch:
                h_tiles.append(alloc_hchunk())
                x_tiles[c + 1] = alloc_xchunk(c + 1)
            if jp == 0 and t > 0:
                xg_cur, xg_nxt = xg_nxt, None
            xg_ps = xg_cur
            if t + 1 < t_steps:
                nxt = h_tiles[(t + 1) // tcs]
                jn = (t + 1) % tcs
                hdst = nxt[:, jn * BL : (jn + 1) * BL]
            else:
                hdst = stub[:, 0:BL]

            rhs_ap = cur[:, j * BL : (j + 1) * BL]
            mm_a_i = nc.tensor.matmul(
                xg_ps[:, 0, jp * BL : (jp + 1) * BL],
                wha_t[:],
                rhs_ap,
                start=False,
                stop=(jp == tcp - 1),
                skip_group_check=True,
            )
            mm_b_i = nc.tensor.matmul(
                xg_ps[:, 1, jp * BL : (jp + 1) * BL],
                whb_t[:],
                rhs_ap,
                start=False,
                stop=(jp == tcp - 1),
                skip_group_check=True,
            )
            # s: cols 0:BL = [sig_i; sig_f], cols BL:2BL = [sig_2g; sig_o]
            s_t = sig_pool.tile([128, 2, BL], F32, tag="s")
            nc.scalar.activation(
                s_t[:], xg_ps[:, :, jp * BL : (jp + 1) * BL], AF.Sigmoid
            )

            r_t = small_pool.tile([128, BL], F32, tag="r")
            nc.vector.tensor_mul(
                r_t[H:128, :], s_t[H:128, 0, :], c_state[H:128, :]
            )  # f*c'
            q_t = small_pool.tile([128, BL], F32, tag="q")
            nc.vector.scalar_tensor_tensor(
                q_t[H:128, :],
                s_t[0:H, 1, :],
                0.5,
                s_t[0:H, 0, :],
                ALU.subtract,
                ALU.mult,
            )  # (sig_2g - 0.5) * i
            c_new = state_pool.tile([128, BL], F32, tag="c")
            nc.vector.tensor_add(c_new[H:128, :], r_t[H:128, :], q_t[H:128, :])
            c_state = c_new

            tc_t = small_pool.tile([128, BL], F32, tag="tc")
            tanh_i = nc.scalar.activation(
                tc_t[H:128, :], c_new[H:128, :], AF.Tanh, scale=2.0
            )
            nc.vector.tensor_mul(hdst, s_t[H:128, 1, :], tc_t[H:128, :])  # o*tanh

            if 0 < jp < tcp - 1:
                # keep the PE HAM window active so matmuls stay at the warm
                # clock; dep-pinned behind this step's gate matmuls so they
                # fill the idle window instead of delaying the next step
                for _w in range(2):
                    wmm_i = nc.tensor.matmul(
                        warm_ps[:], wha_t[:], dummy[:],
                        start=True, stop=True, skip_group_check=True,
                    )
                    add_dep_helper(wmm_i.ins, mm_b_i.ins, False, "pe-warm")
            if jp == 0:
                if t + tcp < t_steps:
                    # prefetch next psum-chunk's x-projection; force it
                    # behind this step's gate matmuls so it fills the idle
                    # PE window instead of delaying the critical path
                    xg_nxt, xg_bis = emit_xg(t + tcp)
                    for bi in xg_bis:
                        add_dep_helper(bi, mm_b_i.ins, False, "fill-idle")
                if t > 0:
                    # head for slice [t-tcp, t): one matmul with replicated
                    # W_out stationary (all psum rows identical); sigmoid
                    # only row 0
                    a0 = t - tcp
                    hc0, hj0 = divmod(a0, tcs)
                    hsl = h_tiles[hc0][:, hj0 * BL : (hj0 + tcp) * BL]
                    hp = hpsum_pool.tile([H, tcp * BL], F32, tag="hp")
                    hmm_i = nc.tensor.matmul(
                        hp[:], wo_t[:], hsl, start=True, stop=True
                    )
                    add_dep_helper(hmm_i.ins, mm_b_i.ins, False, "fill-idle")
                    hb = hd_pool.tile([1, tcp * BL], F32, tag="hb")
                    hsig_i = nc.scalar.activation(
                        hb[:], hp[0:1, :], AF.Sigmoid, bias=bo_t[0:1, 0:1]
                    )
                    add_dep_helper(hsig_i.ins, tanh_i.ins, False, "fill-idle")
                    nc.sync.dma_start(
                        hd[0:1, a0 * BL : (a0 + tcp) * BL], hb[:]
                    )
            if j == tcs - 1:
                nc.sync.dma_start(
                    hs_r[:, c * tcs * BL : (c + 1) * tcs * BL], cur[:, :]
                )

        nc.sync.dma_start(hs_r[:, t_steps * BL : (t_steps + 1) * BL], stub[:, :])
        # head for the last tcp slice
        la0 = t_steps - tcp
        lc0, lj0 = divmod(la0, tcs)
        lsl = h_tiles[lc0][:, lj0 * BL : (lj0 + tcp) * BL]
        lhp = hpsum_pool.tile([H, tcp * BL], F32, tag="hp")
        nc.tensor.matmul(lhp[:], wo_t[:], lsl, start=True, stop=True)
        lhb = hd_pool.tile([1, tcp * BL], F32, tag="hb")
        nc.scalar.activation(
            lhb[:], lhp[0:1, :], AF.Sigmoid, bias=bo_t[0:1, 0:1]
        )
        nc.sync.dma_start(hd[0:1, la0 * BL : (la0 + tcp) * BL], lhb[:])
        # head for the final h^{(T)}
        hps = hpsum_pool.tile([H, BL], F32, tag="hps")
        nc.tensor.matmul(hps[:], wo_t[:], stub[:, :], start=True, stop=True)
        hbs = hd_pool.tile([1, BL], F32, tag="hbs")
        nc.scalar.activation(
            hbs[:], hps[0:1, :], AF.Sigmoid, bias=bo_t[0:1, 0:1]
        )
        nc.sync.dma_start(
            hd[0:1, t_steps * BL : (t_steps + 1) * BL], hbs[:]
        )

    nc.finalize()
    return nc


def make_in_maps(x, h_0, c_0, W_ih, W_hh, b_ih, b_hh, W_out, b_out):
    x = np.asarray(x, np.float32)
    t_steps = x.shape[1]
    nch = (t_steps + TC - 1) // TC
    Wh = np.asarray(W_hh, np.float32).copy()  # [4H, H]
    Wx = np.concatenate(
        [
            np.asarray(W_ih, np.float32),
            (np.asarray(b_ih, np.float32) + np.asarray(b_hh, np.float32))[:, None],
        ],
        axis=1,
    ).copy()  # [4H, 33]
    Wh[2 * H : 3 * H, :] *= 2.0  # g rows: sigmoid(2g) trick
    Wx[2 * H : 3 * H, :] *= 2.0
    wha = np.ascontiguousarray(Wh[0 : 2 * H, :].T)  # [64, 128] (i, f)
    whb = np.ascontiguousarray(Wh[2 * H : 4 * H, :].T)  # [64, 128] (2g, o)
    wxa = np.ascontiguousarray(Wx[0 : 2 * H, :].T)  # [33, 128]
    wxb = np.ascontiguousarray(Wx[2 * H : 4 * H, :].T)  # [33, 128]
    bf = np.float16
    wo = np.ascontiguousarray(
        np.tile(np.asarray(W_out, np.float32).T, (1, H))
    ).astype(bf)  # [H, H]: column m = W_out for every m
    bscalar = float(np.asarray(b_out, np.float32).reshape(-1)[0])

    xa_full = np.concatenate(
        [x, np.ones((x.shape[0], t_steps, 1), np.float32)], axis=2
    )  # [B, T, 33]
    xa_full = np.ascontiguousarray(xa_full.transpose(2, 1, 0))  # [33, T, B]
    h0_full = np.ascontiguousarray(np.asarray(h_0, np.float32)[0].T)  # [H, B]
    c0_full = np.ascontiguousarray(np.asarray(c_0, np.float32)[0].T) * 0.5

    in_maps = []
    for k in range(NCORES):
        sl = slice(k * BL, (k + 1) * BL)
        in_maps.append(
            {
                "xa": np.ascontiguousarray(xa_full[:, :, sl]).astype(bf),
                "h0": np.ascontiguousarray(h0_full[:, sl]).astype(bf),
                "c0h": np.ascontiguousarray(c0_full[:, sl]),
                "wha": wha.astype(bf),
                "whb": whb.astype(bf),
                "wxa": wxa.astype(bf),
                "wxb": wxb.astype(bf),
                "wo": wo,
                "bo": np.full((1, 1), bscalar, np.float32),
            }
        )
    return in_maps


def assemble_outputs(results, t_steps: int = T):
    bsz = NCORES * BL
    tcs = min(TC, t_steps)
    nch = (t_steps + tcs - 1) // tcs
    hs_out = np.empty((bsz, t_steps, H), np.float32)
    out = np.empty((bsz, t_steps, 1), np.float32)
    for k in range(NCORES):
        hs_k = np.asarray(results[k]["hs"]).astype(np.float32)  # [H, T+1, BL]
        hs_out[k * BL : (k + 1) * BL] = hs_k[:, 1:, :].transpose(2, 1, 0)
        hd_k = np.asarray(results[k]["hd"])  # [1, (T+1)*BL]
        heads = hd_k.reshape(t_steps + 1, BL)
        out[k * BL : (k + 1) * BL, :, 0] = heads[1 : t_steps + 1, :].T
    return out, hs_out


def kernel(x, h_0, c_0, W_ih, W_hh, b_ih, b_hh, W_out, b_out):
    import time

    in_maps = make_in_maps(x, h_0, c_0, W_ih, W_hh, b_ih, b_hh, W_out, b_out)
    nc = build_program(T)
    last_err = None
    for attempt in range(3):
        try:
            res = run_bass_kernel_spmd(nc, in_maps, list(range(NCORES))).results
            return assemble_outputs(res, T)
        except Exception as e:  # transient NRT device errors: retry
            last_err = e
            time.sleep(10.0)
    raise last_err


if __name__ == "__main__":
    nc = build_program(T)
    print("build ok")


# revision 30
# speedup vs baseline: 1.0000x; 1.0000x over previous
"""LSTM (B=512, T=512, D=32, H=64) + sigmoid linear head on 8 NeuronCores.

Data-parallel over batch (64 per core); the T=512 recurrence runs locally
per core. Everything lives in transposed [feature, batch] layout so the
per-step matmul contracts over partitions.

Structure (v2):
  - The x-projection (W_ih @ x + biases) is bulk-matmul'ed per 8-step
    chunk straight into PSUM (K=33 incl. a ones-row for the biases);
    the per-step W_hh @ h matmuls then accumulate onto it (start=False),
    so each step needs only 2 small K=64 matmuls on the critical path.
  - g-gate rows of the weights are pre-scaled by 2 so ONE sigmoid op
    covers i/f/o AND tanh(g) (tanh(g) = 2*sigmoid(2g) - 1).
  - cell state is stored halved (c' = c/2):
        c' = f*c' + i*(sigmoid(2g) - 0.5)
    f*c' runs on GpSimd in parallel with the fused scalar_tensor_tensor
    on DVE; tanh(c) = Tanh(2*c') uses the activation's free input scale.
  - h_t is written directly into the next step's matmul-rhs tile (which
    doubles as the hs output staging buffer).
  - The linear head runs on GpSimd per 64-step chunk: per-partition
    scale by W_out then a partition-sum reduce into one accumulator row;
    one sigmoid + DMA at the very end.
"""

from contextlib import ExitStack

import numpy as np

import concourse.bacc as bacc
import concourse.bass as bass
import concourse.mybir as mybir
import concourse.tile as tile
from concourse.bass_utils import run_bass_kernel_spmd
from concourse.tile import add_dep_helper

B, T, D, H = 512, 512, 32, 64
NCORES = 8
BL = B // NCORES  # 64 batch per core
KX = D + 1  # 33: [x; ones]
TC = 64  # timesteps per sbuf chunk tile
TCP = 8  # timesteps per psum xg chunk
F32 = mybir.dt.float32
BF16 = mybir.dt.float16
AF = mybir.ActivationFunctionType
ALU = mybir.AluOpType


def build_program(t_steps: int = T):
    tcs = min(TC, t_steps)
    tcp = min(TCP, t_steps)
    nch = (t_steps + tcs - 1) // tcs
    nc = bacc.Bacc()
    xa = nc.declare_dram_parameter("xa", [KX, t_steps, BL], BF16, False)
    h0 = nc.declare_dram_parameter("h0", [H, BL], BF16, False)
    c0h = nc.declare_dram_parameter("c0h", [H, BL], F32, False)
    wha = nc.declare_dram_parameter("wha", [H, 2 * H], BF16, False)
    whb = nc.declare_dram_parameter("whb", [H, 2 * H], BF16, False)
    wxa = nc.declare_dram_parameter("wxa", [KX, 2 * H], BF16, False)
    wxb = nc.declare_dram_parameter("wxb", [KX, 2 * H], BF16, False)
    wo = nc.declare_dram_parameter("wo", [H, H], BF16, False)
    bo = nc.declare_dram_parameter("bo", [1, 1], F32, False)
    hs = nc.declare_dram_parameter("hs", [H, t_steps + 1, BL], BF16, True)
    hd = nc.declare_dram_parameter("hd", [1, (t_steps + 1) * BL], F32, True)

    xa_r = xa.rearrange("d t b -> d (t b)")
    hs_r = hs.rearrange("h t b -> h (t b)")

    with tile.TileContext(nc) as tc, ExitStack() as ctx:
        const_pool = ctx.enter_context(tc.tile_pool(name="const", bufs=1))
        h_pool = ctx.enter_context(tc.tile_pool(name="hbuf", bufs=4))
        x_pool = ctx.enter_context(tc.tile_pool(name="xbuf", bufs=3))
        hd_pool = ctx.enter_context(tc.tile_pool(name="hdbuf", bufs=2))
        sig_pool = ctx.enter_context(tc.tile_pool(name="sig", bufs=8))
        small_pool = ctx.enter_context(tc.tile_pool(name="small", bufs=8))
        state_pool = ctx.enter_context(tc.tile_pool(name="state", bufs=8))
        gpsum_pool = ctx.enter_context(
            tc.tile_pool(name="gpsum", bufs=2, space="PSUM")
        )
        hpsum_pool = ctx.enter_context(
            tc.tile_pool(name="hpsum", bufs=2, space="PSUM")
        )

        wha_t = const_pool.tile([H, 2 * H], BF16, tag="wha")
        nc.sync.dma_start(wha_t[:], wha[:])
        whb_t = const_pool.tile([H, 2 * H], BF16, tag="whb")
        nc.sync.dma_start(whb_t[:], whb[:])
        wxa_t = const_pool.tile([KX, 2 * H], BF16, tag="wxa")
        nc.sync.dma_start(wxa_t[:], wxa[:])
        wxb_t = const_pool.tile([KX, 2 * H], BF16, tag="wxb")
        nc.sync.dma_start(wxb_t[:], wxb[:])
        wo_t = const_pool.tile([H, H], BF16, tag="wo")
        nc.sync.dma_start(wo_t[:], wo[:])
        bo_t = const_pool.tile([1, 1], F32, tag="bo")
        nc.sync.dma_start(bo_t[:], bo[:])

        def alloc_hchunk():
            ht_ = h_pool.tile([H, tcs * BL], BF16, tag="hchunk")
            return ht_

        def alloc_xchunk(c):
            t_ = x_pool.tile([KX, tcs * BL], BF16, tag="xchunk")
            nc.sync.dma_start(
                t_[:, :], xa_r[:, c * tcs * BL : (c + 1) * tcs * BL]
            )
            return t_

        h_tiles = [alloc_hchunk()]
        x_tiles = {0: alloc_xchunk(0)}
        nc.sync.dma_start(h_tiles[0][:, 0:BL], h0[:])
        stub = hd_pool.tile([H, BL], BF16, tag="stub")

        c_state = state_pool.tile([128, BL], F32, tag="c")
        nc.sync.dma_start(c_state[H:128, :], c0h[:])

        def emit_xg(tstart):
            # bulk x-projection for steps [tstart, tstart+tcp) into PSUM
            xg_ = gpsum_pool.tile([128, 2, tcp * BL], F32, tag="xg")
            xc, xj = divmod(tstart, tcs)
            xsl = x_tiles[xc][:, xj * BL : xj * BL + tcp * BL]
            b0 = nc.tensor.matmul(
                xg_[:, 0, :], wxa_t[:], xsl, start=True, stop=False
            )
            b1 = nc.tensor.matmul(
                xg_[:, 1, :], wxb_t[:], xsl, start=True, stop=False
            )
            return xg_, [b0.ins, b1.ins]

        xg_cur, _ = emit_xg(0)
        xg_nxt = None
        for t in range(t_steps):
            c, j = divmod(t, tcs)
            cp, jp = divmod(t, tcp)
            cur = h_tiles[c]
            if j == 0 and c + 1 < nch:
                h_tiles.append(alloc_hchunk())
                x_tiles[c + 1] = alloc_xchunk(c + 1)
            if jp == 0 and t > 0:
                xg_cur, xg_nxt = xg_nxt, None
            xg_ps = xg_cur
            if t + 1 < t_steps:
                nxt = h_tiles[(t + 1) // tcs]
                jn = (t + 1) % tcs
                hdst = nxt[:, jn * BL : (jn + 1) * BL]
            else:
                hdst = stub[:, 0:BL]

            rhs_ap = cur[:, j * BL : (j + 1) * BL]
            mm_a_i = nc.tensor.matmul(
                xg_ps[:, 0, jp * BL : (jp + 1) * BL],
                wha_t[:],
                rhs_ap,
                start=False,
                stop=(jp == tcp - 1),
                skip_group_check=True,
            )
            mm_b_i = nc.tensor.matmul(
                xg_ps[:, 1, jp * BL : (jp + 1) * BL],
                whb_t[:],
                rhs_ap,
                start=False,
                stop=(jp == tcp - 1),
                skip_group_check=True,
            )
            # s: cols 0:BL = [sig_i; sig_f], cols BL:2BL = [sig_2g; sig_o]
            s_t = sig_pool.tile([128, 2, BL], F32, tag="s")
            nc.scalar.activation(
                s_t[:], xg_ps[:, :, jp * BL : (jp + 1) * BL], AF.Sigmoid
            )

            r_t = small_pool.tile([128, BL], F32, tag="r")
            nc.vector.tensor_mul(
                r_t[H:128, :], s_t[H:128, 0, :], c_state[H:128, :]
            )  # f*c'
            q_t = small_pool.tile([128, BL], F32, tag="q")
            nc.vector.scalar_tensor_tensor(
                q_t[H:128, :],
                s_t[0:H, 1, :],
                0.5,
                s_t[0:H, 0, :],
                ALU.subtract,
                ALU.mult,
            )  # (sig_2g - 0.5) * i
            c_new = state_pool.tile([128, BL], F32, tag="c")
            nc.vector.tensor_add(c_new[H:128, :], r_t[H:128, :], q_t[H:128, :])
            c_state = c_new

            tc_t = small_pool.tile([128, BL], F32, tag="tc")
            tanh_i = nc.scalar.activation(
                tc_t[H:128, :], c_new[H:128, :], AF.Tanh, scale=2.0
            )
            nc.vector.tensor_mul(hdst, s_t[H:128, 1, :], tc_t[H:128, :])  # o*tanh

            if jp == 0:
                if t + tcp < t_steps:
                    # prefetch next psum-chunk's x-projection; force it
                    # behind this step's gate matmuls so it fills the idle
                    # PE window instead of delaying the critical path
                    xg_nxt, xg_bis = emit_xg(t + tcp)
                    for bi in xg_bis:
                        add_dep_helper(bi, mm_b_i.ins, False, "fill-idle")
                if t > 0:
                    # head for slice [t-tcp, t): one matmul with replicated
                    # W_out stationary (all psum rows identical); sigmoid
                    # only row 0
                    a0 = t - tcp
                    hc0, hj0 = divmod(a0, tcs)
                    hsl = h_tiles[hc0][:, hj0 * BL : (hj0 + tcp) * BL]
                    hp = hpsum_pool.tile([H, tcp * BL], F32, tag="hp")
                    hmm_i = nc.tensor.matmul(
                        hp[:], wo_t[:], hsl, start=True, stop=True
                    )
                    add_dep_helper(hmm_i.ins, mm_b_i.ins, False, "fill-idle")
                    hb = hd_pool.tile([1, tcp * BL], F32, tag="hb")
                    hsig_i = nc.scalar.activation(
                        hb[:], hp[0:1, :], AF.Sigmoid, bias=bo_t[0:1, 0:1]
                    )
                    add_dep_helper(hsig_i.ins, tanh_i.ins, False, "fill-idle")
                    nc.sync.dma_start(
                        hd[0:1, a0 * BL : (a0 + tcp) * BL], hb[:]
                    )
            if j == tcs - 1:
                nc.sync.dma_start(
                    hs_r[:, c * tcs * BL : (c + 1) * tcs * BL], cur[:, :]
                )

        nc.sync.dma_start(hs_r[:, t_steps * BL : (t_steps + 1) * BL], stub[:, :])
        # head for the last tcp slice
        la0 = t_steps - tcp
        lc0, lj0 = divmod(la0, tcs)
        lsl = h_tiles[lc0][:, lj0 * BL : (lj0 + tcp) * BL]
        lhp = hpsum_pool.tile([H, tcp * BL], F32, tag="hp")
        nc.tensor.matmul(lhp[:], wo_t[:], lsl, start=True, stop=True)
        lhb = hd_pool.tile([1, tcp * BL], F32, tag="hb")
        nc.scalar.activation(
            lhb[:], lhp[0:1, :], AF.Sigmoid, bias=bo_t[0:1, 0:1]
        )
        nc.sync.dma_start(hd[0:1, la0 * BL : (la0 + tcp) * BL], lhb[:])
        # head for the final h^{(T)}
        hps = hpsum_pool.tile([H, BL], F32, tag="hps")
        nc.tensor.matmul(hps[:], wo_t[:], stub[:, :], start=True, stop=True)
        hbs = hd_pool.tile([1, BL], F32, tag="hbs")
        nc.scalar.activation(
            hbs[:], hps[0:1, :], AF.Sigmoid, bias=bo_t[0:1, 0:1]
        )
        nc.sync.dma_start(
            hd[0:1, t_steps * BL : (t_steps + 1) * BL], hbs[:]
        )

    nc.finalize()
    return nc


def make_in_maps(x, h_0, c_0, W_ih, W_hh, b_ih, b_hh, W_out, b_out):
    x = np.asarray(x, np.float32)
    t_steps = x.shape[1]
    nch = (t_steps + TC - 1) // TC
    Wh = np.asarray(W_hh, np.float32).copy()  # [4H, H]
    Wx = np.concatenate(
        [
            np.asarray(W_ih, np.float32),
            (np.asarray(b_ih, np.float32) + np.asarray(b_hh, np.float32))[:, None],
        ],
        axis=1,
    ).copy()  # [4H, 33]
    Wh[2 * H : 3 * H, :] *= 2.0  # g rows: sigmoid(2g) trick
    Wx[2 * H : 3 * H, :] *= 2.0
    wha = np.ascontiguousarray(Wh[0 : 2 * H, :].T)  # [64, 128] (i, f)
    whb = np.ascontiguousarray(Wh[2 * H : 4 * H, :].T)  # [64, 128] (2g, o)
    wxa = np.ascontiguousarray(Wx[0 : 2 * H, :].T)  # [33, 128]
    wxb = np.ascontiguousarray(Wx[2 * H : 4 * H, :].T)  # [33, 128]
    bf = np.float16
    wo = np.ascontiguousarray(
        np.tile(np.asarray(W_out, np.float32).T, (1, H))
    ).astype(bf)  # [H, H]: column m = W_out for every m
    bscalar = float(np.asarray(b_out, np.float32).reshape(-1)[0])

    xa_full = np.concatenate(
        [x, np.ones((x.shape[0], t_steps, 1), np.float32)], axis=2
    )  # [B, T, 33]
    xa_full = np.ascontiguousarray(xa_full.transpose(2, 1, 0))  # [33, T, B]
    h0_full = np.ascontiguousarray(np.asarray(h_0, np.float32)[0].T)  # [H, B]
    c0_full = np.ascontiguousarray(np.asarray(c_0, np.float32)[0].T) * 0.5

    in_maps = []
    for k in range(NCORES):
        sl = slice(k * BL, (k + 1) * BL)
        in_maps.append(
            {
                "xa": np.ascontiguousarray(xa_full[:, :, sl]).astype(bf),
                "h0": np.ascontiguousarray(h0_full[:, sl]).astype(bf),
                "c0h": np.ascontiguousarray(c0_full[:, sl]),
                "wha": wha.astype(bf),
                "whb": whb.astype(bf),
                "wxa": wxa.astype(bf),
                "wxb": wxb.astype(bf),
                "wo": wo,
                "bo": np.full((1, 1), bscalar, np.float32),
            }
        )
    return in_maps


def assemble_outputs(results, t_steps: int = T):
    bsz = NCORES * BL
    tcs = min(TC, t_steps)
    nch = (t_steps + tcs - 1) // tcs
    hs_out = np.empty((bsz, t_steps, H), np.float32)
    out = np.empty((bsz, t_steps, 1), np.float32)
    for k in range(NCORES):
        hs_k = np.asarray(results[k]["hs"]).astype(np.float32)  # [H, T+1, BL]
        hs_out[k * BL : (k + 1) * BL] = hs_k[:, 1:, :].transpose(2, 1, 0)
        hd_k = np.asarray(results[k]["hd"])  # [1, (T+1)*BL]
        heads = hd_k.reshape(t_steps + 1, BL)
        out[k * BL : (k + 1) * BL, :, 0] = heads[1 : t_steps + 1, :].T
    return out, hs_out


def kernel(x, h_0, c_0, W_ih, W_hh, b_ih, b_hh, W_out, b_out):
    import time

    in_maps = make_in_maps(x, h_0, c_0, W_ih, W_hh, b_ih, b_hh, W_out, b_out)
    nc = build_program(T)
    last_err = None
    for attempt in range(3):
        try:
            res = run_bass_kernel_spmd(nc, in_maps, list(range(NCORES))).results
            return assemble_outputs(res, T)
        except Exception as e:  # transient NRT device errors: retry
            last_err = e
            time.sleep(10.0)
    raise last_err


if __name__ == "__main__":
    nc = build_program(T)
    print("build ok")


# revision 31
# speedup vs baseline: 1.0462x; 1.0462x over previous
"""LSTM (B=512, T=512, D=32, H=64) + sigmoid linear head on 8 NeuronCores.

Data-parallel over batch (64 per core); the T=512 recurrence runs locally
per core. Everything lives in transposed [feature, batch] layout so the
per-step matmul contracts over partitions.

Structure (v2):
  - The x-projection (W_ih @ x + biases) is bulk-matmul'ed per 8-step
    chunk straight into PSUM (K=33 incl. a ones-row for the biases);
    the per-step W_hh @ h matmuls then accumulate onto it (start=False),
    so each step needs only 2 small K=64 matmuls on the critical path.
  - g-gate rows of the weights are pre-scaled by 2 so ONE sigmoid op
    covers i/f/o AND tanh(g) (tanh(g) = 2*sigmoid(2g) - 1).
  - cell state is stored halved (c' = c/2):
        c' = f*c' + i*(sigmoid(2g) - 0.5)
    f*c' runs on GpSimd in parallel with the fused scalar_tensor_tensor
    on DVE; tanh(c) = Tanh(2*c') uses the activation's free input scale.
  - h_t is written directly into the next step's matmul-rhs tile (which
    doubles as the hs output staging buffer).
  - The linear head runs on GpSimd per 64-step chunk: per-partition
    scale by W_out then a partition-sum reduce into one accumulator row;
    one sigmoid + DMA at the very end.
"""

from contextlib import ExitStack

import numpy as np

import concourse.bacc as bacc
import concourse.bass as bass
import concourse.mybir as mybir
import concourse.tile as tile
from concourse.bass_utils import run_bass_kernel_spmd
from concourse.tile import add_dep_helper

B, T, D, H = 512, 512, 32, 64
NCORES = 8
BL = B // NCORES  # 64 batch per core
KX = D + 1  # 33: [x; ones]
TC = 64  # timesteps per sbuf chunk tile
TCP = 8  # timesteps per psum xg chunk
F32 = mybir.dt.float32
BF16 = mybir.dt.float16
AF = mybir.ActivationFunctionType
ALU = mybir.AluOpType


def build_program(t_steps: int = T):
    tcs = min(TC, t_steps)
    tcp = min(TCP, t_steps)
    nch = (t_steps + tcs - 1) // tcs
    nc = bacc.Bacc()
    xa = nc.declare_dram_parameter("xa", [KX, t_steps, BL], BF16, False)
    h0 = nc.declare_dram_parameter("h0", [H, BL], BF16, False)
    c0h = nc.declare_dram_parameter("c0h", [H, BL], BF16, False)
    wha = nc.declare_dram_parameter("wha", [H, 2 * H], BF16, False)
    whb = nc.declare_dram_parameter("whb", [H, 2 * H], BF16, False)
    wxa = nc.declare_dram_parameter("wxa", [KX, 2 * H], BF16, False)
    wxb = nc.declare_dram_parameter("wxb", [KX, 2 * H], BF16, False)
    wo = nc.declare_dram_parameter("wo", [H, H], BF16, False)
    bo = nc.declare_dram_parameter("bo", [1, 1], F32, False)
    hs = nc.declare_dram_parameter("hs", [H, t_steps + 1, BL], BF16, True)
    hd = nc.declare_dram_parameter("hd", [1, (t_steps + 1) * BL], F32, True)

    xa_r = xa.rearrange("d t b -> d (t b)")
    hs_r = hs.rearrange("h t b -> h (t b)")

    with tile.TileContext(nc) as tc, ExitStack() as ctx:
        const_pool = ctx.enter_context(tc.tile_pool(name="const", bufs=1))
        h_pool = ctx.enter_context(tc.tile_pool(name="hbuf", bufs=4))
        x_pool = ctx.enter_context(tc.tile_pool(name="xbuf", bufs=3))
        hd_pool = ctx.enter_context(tc.tile_pool(name="hdbuf", bufs=2))
        sig_pool = ctx.enter_context(tc.tile_pool(name="sig", bufs=8))
        small_pool = ctx.enter_context(tc.tile_pool(name="small", bufs=8))
        state_pool = ctx.enter_context(tc.tile_pool(name="state", bufs=8))
        gpsum_pool = ctx.enter_context(
            tc.tile_pool(name="gpsum", bufs=2, space="PSUM")
        )
        hpsum_pool = ctx.enter_context(
            tc.tile_pool(name="hpsum", bufs=2, space="PSUM")
        )

        wha_t = const_pool.tile([H, 2 * H], BF16, tag="wha")
        nc.sync.dma_start(wha_t[:], wha[:])
        whb_t = const_pool.tile([H, 2 * H], BF16, tag="whb")
        nc.sync.dma_start(whb_t[:], whb[:])
        wxa_t = const_pool.tile([KX, 2 * H], BF16, tag="wxa")
        nc.sync.dma_start(wxa_t[:], wxa[:])
        wxb_t = const_pool.tile([KX, 2 * H], BF16, tag="wxb")
        nc.sync.dma_start(wxb_t[:], wxb[:])
        wo_t = const_pool.tile([H, H], BF16, tag="wo")
        nc.sync.dma_start(wo_t[:], wo[:])
        bo_t = const_pool.tile([1, 1], F32, tag="bo")
        nc.sync.dma_start(bo_t[:], bo[:])

        def alloc_hchunk():
            ht_ = h_pool.tile([H, tcs * BL], BF16, tag="hchunk")
            return ht_

        def alloc_xchunk(c):
            t_ = x_pool.tile([KX, tcs * BL], BF16, tag="xchunk")
            nc.sync.dma_start(
                t_[:, :], xa_r[:, c * tcs * BL : (c + 1) * tcs * BL]
            )
            return t_

        h_tiles = [alloc_hchunk()]
        x_tiles = {0: alloc_xchunk(0)}
        nc.sync.dma_start(h_tiles[0][:, 0:BL], h0[:])
        stub = hd_pool.tile([H, BL], BF16, tag="stub")

        c_state = state_pool.tile([128, BL], BF16, tag="c")
        nc.sync.dma_start(c_state[H:128, :], c0h[:])

        def emit_xg(tstart):
            # bulk x-projection for steps [tstart, tstart+tcp) into PSUM
            xg_ = gpsum_pool.tile([128, 2, tcp * BL], F32, tag="xg")
            xc, xj = divmod(tstart, tcs)
            xsl = x_tiles[xc][:, xj * BL : xj * BL + tcp * BL]
            b0 = nc.tensor.matmul(
                xg_[:, 0, :], wxa_t[:], xsl, start=True, stop=False
            )
            b1 = nc.tensor.matmul(
                xg_[:, 1, :], wxb_t[:], xsl, start=True, stop=False
            )
            return xg_, [b0.ins, b1.ins]

        xg_cur, _ = emit_xg(0)
        xg_nxt = None
        for t in range(t_steps):
            c, j = divmod(t, tcs)
            cp, jp = divmod(t, tcp)
            cur = h_tiles[c]
            if j == 0 and c + 1 < nch:
                h_tiles.append(alloc_hchunk())
                x_tiles[c + 1] = alloc_xchunk(c + 1)
            if jp == 0 and t > 0:
                xg_cur, xg_nxt = xg_nxt, None
            xg_ps = xg_cur
            if t + 1 < t_steps:
                nxt = h_tiles[(t + 1) // tcs]
                jn = (t + 1) % tcs
                hdst = nxt[:, jn * BL : (jn + 1) * BL]
            else:
                hdst = stub[:, 0:BL]

            rhs_ap = cur[:, j * BL : (j + 1) * BL]
            mm_a_i = nc.tensor.matmul(
                xg_ps[:, 0, jp * BL : (jp + 1) * BL],
                wha_t[:],
                rhs_ap,
                start=False,
                stop=(jp == tcp - 1),
                skip_group_check=True,
            )
            mm_b_i = nc.tensor.matmul(
                xg_ps[:, 1, jp * BL : (jp + 1) * BL],
                whb_t[:],
                rhs_ap,
                start=False,
                stop=(jp == tcp - 1),
                skip_group_check=True,
            )
            # s: cols 0:BL = [sig_i; sig_f], cols BL:2BL = [sig_2g; sig_o]
            s_t = sig_pool.tile([128, 2, BL], BF16, tag="s")
            nc.scalar.activation(
                s_t[:], xg_ps[:, :, jp * BL : (jp + 1) * BL], AF.Sigmoid
            )

            r_t = small_pool.tile([128, BL], BF16, tag="r")
            nc.vector.tensor_mul(
                r_t[H:128, :], s_t[H:128, 0, :], c_state[H:128, :]
            )  # f*c'
            q_t = small_pool.tile([128, BL], BF16, tag="q")
            nc.vector.scalar_tensor_tensor(
                q_t[H:128, :],
                s_t[0:H, 1, :],
                0.5,
                s_t[0:H, 0, :],
                ALU.subtract,
                ALU.mult,
            )  # (sig_2g - 0.5) * i
            c_new = state_pool.tile([128, BL], BF16, tag="c")
            nc.vector.tensor_add(c_new[H:128, :], r_t[H:128, :], q_t[H:128, :])
            c_state = c_new

            tc_t = small_pool.tile([128, BL], BF16, tag="tc")
            tanh_i = nc.scalar.activation(
                tc_t[H:128, :], c_new[H:128, :], AF.Tanh, scale=2.0
            )
            nc.vector.tensor_mul(hdst, s_t[H:128, 1, :], tc_t[H:128, :])  # o*tanh

            if jp == 0:
                if t + tcp < t_steps:
                    # prefetch next psum-chunk's x-projection; force it
                    # behind this step's gate matmuls so it fills the idle
                    # PE window instead of delaying the critical path
                    xg_nxt, xg_bis = emit_xg(t + tcp)
                    for bi in xg_bis:
                        add_dep_helper(bi, mm_b_i.ins, False, "fill-idle")
                if t > 0:
                    # head for slice [t-tcp, t): one matmul with replicated
                    # W_out stationary (all psum rows identical); sigmoid
                    # only row 0
                    a0 = t - tcp
                    hc0, hj0 = divmod(a0, tcs)
                    hsl = h_tiles[hc0][:, hj0 * BL : (hj0 + tcp) * BL]
                    hp = hpsum_pool.tile([H, tcp * BL], F32, tag="hp")
                    hmm_i = nc.tensor.matmul(
                        hp[:], wo_t[:], hsl, start=True, stop=True
                    )
                    add_dep_helper(hmm_i.ins, mm_b_i.ins, False, "fill-idle")
                    hb = hd_pool.tile([1, tcp * BL], F32, tag="hb")
                    hsig_i = nc.scalar.activation(
                        hb[:], hp[0:1, :], AF.Sigmoid, bias=bo_t[0:1, 0:1]
                    )
                    add_dep_helper(hsig_i.ins, tanh_i.ins, False, "fill-idle")
                    nc.sync.dma_start(
                        hd[0:1, a0 * BL : (a0 + tcp) * BL], hb[:]
                    )
            if j == tcs - 1:
                nc.sync.dma_start(
                    hs_r[:, c * tcs * BL : (c + 1) * tcs * BL], cur[:, :]
                )

        nc.sync.dma_start(hs_r[:, t_steps * BL : (t_steps + 1) * BL], stub[:, :])
        # head for the last tcp slice
        la0 = t_steps - tcp
        lc0, lj0 = divmod(la0, tcs)
        lsl = h_tiles[lc0][:, lj0 * BL : (lj0 + tcp) * BL]
        lhp = hpsum_pool.tile([H, tcp * BL], F32, tag="hp")
        nc.tensor.matmul(lhp[:], wo_t[:], lsl, start=True, stop=True)
        lhb = hd_pool.tile([1, tcp * BL], F32, tag="hb")
        nc.scalar.activation(
            lhb[:], lhp[0:1, :], AF.Sigmoid, bias=bo_t[0:1, 0:1]
        )
        nc.sync.dma_start(hd[0:1, la0 * BL : (la0 + tcp) * BL], lhb[:])
        # head for the final h^{(T)}
        hps = hpsum_pool.tile([H, BL], F32, tag="hps")
        nc.tensor.matmul(hps[:], wo_t[:], stub[:, :], start=True, stop=True)
        hbs = hd_pool.tile([1, BL], F32, tag="hbs")
        nc.scalar.activation(
            hbs[:], hps[0:1, :], AF.Sigmoid, bias=bo_t[0:1, 0:1]
        )
        nc.sync.dma_start(
            hd[0:1, t_steps * BL : (t_steps + 1) * BL], hbs[:]
        )

    nc.finalize()
    return nc


def make_in_maps(x, h_0, c_0, W_ih, W_hh, b_ih, b_hh, W_out, b_out):
    x = np.asarray(x, np.float32)
    t_steps = x.shape[1]
    nch = (t_steps + TC - 1) // TC
    Wh = np.asarray(W_hh, np.float32).copy()  # [4H, H]
    Wx = np.concatenate(
        [
            np.asarray(W_ih, np.float32),
            (np.asarray(b_ih, np.float32) + np.asarray(b_hh, np.float32))[:, None],
        ],
        axis=1,
    ).copy()  # [4H, 33]
    Wh[2 * H : 3 * H, :] *= 2.0  # g rows: sigmoid(2g) trick
    Wx[2 * H : 3 * H, :] *= 2.0
    wha = np.ascontiguousarray(Wh[0 : 2 * H, :].T)  # [64, 128] (i, f)
    whb = np.ascontiguousarray(Wh[2 * H : 4 * H, :].T)  # [64, 128] (2g, o)
    wxa = np.ascontiguousarray(Wx[0 : 2 * H, :].T)  # [33, 128]
    wxb = np.ascontiguousarray(Wx[2 * H : 4 * H, :].T)  # [33, 128]
    bf = np.float16
    wo = np.ascontiguousarray(
        np.tile(np.asarray(W_out, np.float32).T, (1, H))
    ).astype(bf)  # [H, H]: column m = W_out for every m
    bscalar = float(np.asarray(b_out, np.float32).reshape(-1)[0])

    xa_full = np.concatenate(
        [x, np.ones((x.shape[0], t_steps, 1), np.float32)], axis=2
    )  # [B, T, 33]
    xa_full = np.ascontiguousarray(xa_full.transpose(2, 1, 0))  # [33, T, B]
    h0_full = np.ascontiguousarray(np.asarray(h_0, np.float32)[0].T)  # [H, B]
    c0_full = np.ascontiguousarray(np.asarray(c_0, np.float32)[0].T) * 0.5

    in_maps = []
    for k in range(NCORES):
        sl = slice(k * BL, (k + 1) * BL)
        in_maps.append(
            {
                "xa": np.ascontiguousarray(xa_full[:, :, sl]).astype(bf),
                "h0": np.ascontiguousarray(h0_full[:, sl]).astype(bf),
                "c0h": np.ascontiguousarray(c0_full[:, sl]).astype(bf),
                "wha": wha.astype(bf),
                "whb": whb.astype(bf),
                "wxa": wxa.astype(bf),
                "wxb": wxb.astype(bf),
                "wo": wo,
                "bo": np.full((1, 1), bscalar, np.float32),
            }
        )
    return in_maps


def assemble_outputs(results, t_steps: int = T):
    bsz = NCORES * BL
    tcs = min(TC, t_steps)
    nch = (t_steps + tcs - 1) // tcs
    hs_out = np.empty((bsz, t_steps, H), np.float32)
    out = np.empty((bsz, t_steps, 1), np.float32)
    for k in range(NCORES):
        hs_k = np.asarray(results[k]["hs"]).astype(np.float32)  # [H, T+1, BL]
        hs_out[k * BL : (k + 1) * BL] = hs_k[:, 1:, :].transpose(2, 1, 0)
        hd_k = np.asarray(results[k]["hd"])  # [1, (T+1)*BL]
        heads = hd_k.reshape(t_steps + 1, BL)
        out[k * BL : (k + 1) * BL, :, 0] = heads[1 : t_steps + 1, :].T
    return out, hs_out


def kernel(x, h_0, c_0, W_ih, W_hh, b_ih, b_hh, W_out, b_out):
    import time

    in_maps = make_in_maps(x, h_0, c_0, W_ih, W_hh, b_ih, b_hh, W_out, b_out)
    nc = build_program(T)
    last_err = None
    for attempt in range(3):
        try:
            res = run_bass_kernel_spmd(nc, in_maps, list(range(NCORES))).results
            return assemble_outputs(res, T)
        except Exception as e:  # transient NRT device errors: retry
            last_err = e
            time.sleep(10.0)
    raise last_err


if __name__ == "__main__":
    nc = build_program(T)
    print("build ok")


# revision 32
# speedup vs baseline: 1.0484x; 1.0021x over previous
"""LSTM (B=512, T=512, D=32, H=64) + sigmoid linear head on 8 NeuronCores.

Data-parallel over batch (64 per core); the T=512 recurrence runs locally
per core. Everything lives in transposed [feature, batch] layout so the
per-step matmul contracts over partitions.

Structure:
  - The x-projection (W_ih @ x + biases) is bulk-matmul'ed per 8-step
    chunk straight into PSUM (K=33 incl. a ones-row for the biases);
    the per-step W_hh @ h matmuls then accumulate onto it (start=False),
    so each step needs only 2 small K=64 matmuls on the critical path.
  - g-gate rows of the weights are pre-scaled by 2 so ONE sigmoid op
    covers i/f/o AND tanh(g) (tanh(g) = 2*sigmoid(2g) - 1).
  - cell state is stored halved (c' = c/2):
        c' = f*c' + i*(sigmoid(2g) - 0.5)
    computed as one fused scalar_tensor_tensor plus a mul and an add on
    DVE; tanh(c) = Tanh(2*c') uses the activation's free input scale.
    Matmul operands and eltwise state are fp16 (fp32 matmuls secretly run
    as TWO hardware passes; 16-bit DVE ops get the 2x packed mode), with
    fp32 PSUM accumulation throughout.
  - h_t is written directly into the next step's matmul-rhs tile (which
    doubles as the hs output staging buffer).
  - The linear head: one matmul per 8 steps with a replicated-W_out
    stationary (all psum rows identical), sigmoid on psum row 0 only,
    2KB DMA out — dep-pinned behind the step's gate matmuls so it fills
    idle PE/ACT slots instead of delaying the recurrence.
"""

from contextlib import ExitStack

import numpy as np

import concourse.bacc as bacc
import concourse.bass as bass
import concourse.mybir as mybir
import concourse.tile as tile
from concourse.bass_utils import run_bass_kernel_spmd
from concourse.tile import add_dep_helper

B, T, D, H = 512, 512, 32, 64
NCORES = 8
BL = B // NCORES  # 64 batch per core
KX = D + 1  # 33: [x; ones]
TC = 64  # timesteps per sbuf chunk tile
TCP = 8  # timesteps per psum xg chunk
F32 = mybir.dt.float32
BF16 = mybir.dt.float16
AF = mybir.ActivationFunctionType
ALU = mybir.AluOpType


def build_program(t_steps: int = T):
    tcs = min(TC, t_steps)
    tcp = min(TCP, t_steps)
    nch = (t_steps + tcs - 1) // tcs
    nc = bacc.Bacc()
    xa = nc.declare_dram_parameter("xa", [KX, t_steps, BL], BF16, False)
    h0 = nc.declare_dram_parameter("h0", [H, BL], BF16, False)
    c0h = nc.declare_dram_parameter("c0h", [H, BL], BF16, False)
    wha = nc.declare_dram_parameter("wha", [H, 2 * H], BF16, False)
    whb = nc.declare_dram_parameter("whb", [H, 2 * H], BF16, False)
    wxa = nc.declare_dram_parameter("wxa", [KX, 2 * H], BF16, False)
    wxb = nc.declare_dram_parameter("wxb", [KX, 2 * H], BF16, False)
    wo = nc.declare_dram_parameter("wo", [H, H], BF16, False)
    bo = nc.declare_dram_parameter("bo", [1, 1], F32, False)
    hs = nc.declare_dram_parameter("hs", [H, t_steps + 1, BL], BF16, True)
    hd = nc.declare_dram_parameter("hd", [1, (t_steps + 1) * BL], F32, True)

    xa_r = xa.rearrange("d t b -> d (t b)")
    hs_r = hs.rearrange("h t b -> h (t b)")

    with tile.TileContext(nc) as tc, ExitStack() as ctx:
        const_pool = ctx.enter_context(tc.tile_pool(name="const", bufs=1))
        h_pool = ctx.enter_context(tc.tile_pool(name="hbuf", bufs=4))
        x_pool = ctx.enter_context(tc.tile_pool(name="xbuf", bufs=3))
        hd_pool = ctx.enter_context(tc.tile_pool(name="hdbuf", bufs=2))
        sig_pool = ctx.enter_context(tc.tile_pool(name="sig", bufs=8))
        small_pool = ctx.enter_context(tc.tile_pool(name="small", bufs=8))
        state_pool = ctx.enter_context(tc.tile_pool(name="state", bufs=8))
        gpsum_pool = ctx.enter_context(
            tc.tile_pool(name="gpsum", bufs=2, space="PSUM")
        )
        hpsum_pool = ctx.enter_context(
            tc.tile_pool(name="hpsum", bufs=2, space="PSUM")
        )

        wha_t = const_pool.tile([H, 2 * H], BF16, tag="wha")
        nc.sync.dma_start(wha_t[:], wha[:])
        whb_t = const_pool.tile([H, 2 * H], BF16, tag="whb")
        nc.sync.dma_start(whb_t[:], whb[:])
        wxa_t = const_pool.tile([KX, 2 * H], BF16, tag="wxa")
        nc.sync.dma_start(wxa_t[:], wxa[:])
        wxb_t = const_pool.tile([KX, 2 * H], BF16, tag="wxb")
        nc.sync.dma_start(wxb_t[:], wxb[:])
        wo_t = const_pool.tile([H, H], BF16, tag="wo")
        nc.sync.dma_start(wo_t[:], wo[:])
        bo_t = const_pool.tile([1, 1], F32, tag="bo")
        nc.sync.dma_start(bo_t[:], bo[:])

        def alloc_hchunk():
            ht_ = h_pool.tile([H, tcs * BL], BF16, tag="hchunk")
            return ht_

        def alloc_xchunk(c):
            t_ = x_pool.tile([KX, tcs * BL], BF16, tag="xchunk")
            nc.sync.dma_start(
                t_[:, :], xa_r[:, c * tcs * BL : (c + 1) * tcs * BL]
            )
            return t_

        h_tiles = [alloc_hchunk()]
        x_tiles = {0: alloc_xchunk(0)}
        nc.sync.dma_start(h_tiles[0][:, 0:BL], h0[:])
        stub = hd_pool.tile([H, BL], BF16, tag="stub")

        c_state = state_pool.tile([128, BL], BF16, tag="c")
        nc.sync.dma_start(c_state[H:128, :], c0h[:])

        def emit_xg(tstart):
            # bulk x-projection for steps [tstart, tstart+tcp) into PSUM
            xg_ = gpsum_pool.tile([128, 2, tcp * BL], F32, tag="xg")
            xc, xj = divmod(tstart, tcs)
            xsl = x_tiles[xc][:, xj * BL : xj * BL + tcp * BL]
            b0 = nc.tensor.matmul(
                xg_[:, 0, :], wxa_t[:], xsl, start=True, stop=False
            )
            b1 = nc.tensor.matmul(
                xg_[:, 1, :], wxb_t[:], xsl, start=True, stop=False
            )
            return xg_, [b0.ins, b1.ins]

        xg_cur, _ = emit_xg(0)
        xg_nxt = None
        for t in range(t_steps):
            c, j = divmod(t, tcs)
            cp, jp = divmod(t, tcp)
            cur = h_tiles[c]
            if j == 0 and c + 1 < nch:
                h_tiles.append(alloc_hchunk())
                x_tiles[c + 1] = alloc_xchunk(c + 1)
            if jp == 0 and t > 0:
                xg_cur, xg_nxt = xg_nxt, None
            xg_ps = xg_cur
            if t + 1 < t_steps:
                nxt = h_tiles[(t + 1) // tcs]
                jn = (t + 1) % tcs
                hdst = nxt[:, jn * BL : (jn + 1) * BL]
            else:
                hdst = stub[:, 0:BL]

            rhs_ap = cur[:, j * BL : (j + 1) * BL]
            mm_a_i = nc.tensor.matmul(
                xg_ps[:, 0, jp * BL : (jp + 1) * BL],
                wha_t[:],
                rhs_ap,
                start=False,
                stop=(jp == tcp - 1),
                skip_group_check=True,
            )
            mm_b_i = nc.tensor.matmul(
                xg_ps[:, 1, jp * BL : (jp + 1) * BL],
                whb_t[:],
                rhs_ap,
                start=False,
                stop=(jp == tcp - 1),
                skip_group_check=True,
            )
            # s: cols 0:BL = [sig_i; sig_f], cols BL:2BL = [sig_2g; sig_o]
            s_t = sig_pool.tile([128, 2, BL], BF16, tag="s")
            nc.scalar.activation(
                s_t[:], xg_ps[:, :, jp * BL : (jp + 1) * BL], AF.Sigmoid
            )

            r_t = small_pool.tile([128, BL], BF16, tag="r")
            nc.vector.tensor_mul(
                r_t[H:128, :], s_t[H:128, 0, :], c_state[H:128, :]
            )  # f*c'
            q_t = small_pool.tile([128, BL], BF16, tag="q")
            nc.vector.scalar_tensor_tensor(
                q_t[H:128, :],
                s_t[0:H, 1, :],
                0.5,
                s_t[0:H, 0, :],
                ALU.subtract,
                ALU.mult,
            )  # (sig_2g - 0.5) * i
            c_new = state_pool.tile([128, BL], BF16, tag="c")
            nc.vector.tensor_add(c_new[H:128, :], r_t[H:128, :], q_t[H:128, :])
            c_state = c_new

            tc_t = small_pool.tile([128, BL], BF16, tag="tc")
            tanh_i = nc.scalar.activation(
                tc_t[H:128, :], c_new[H:128, :], AF.Tanh, scale=2.0
            )
            nc.vector.tensor_mul(hdst, s_t[H:128, 1, :], tc_t[H:128, :])  # o*tanh

            if jp == 0:
                if t + tcp < t_steps:
                    # prefetch next psum-chunk's x-projection; force it
                    # behind this step's gate matmuls so it fills the idle
                    # PE window instead of delaying the critical path
                    xg_nxt, xg_bis = emit_xg(t + tcp)
                    for bi in xg_bis:
                        add_dep_helper(bi, mm_b_i.ins, False, "fill-idle")
                if t > 0:
                    # head for slice [t-tcp, t): one matmul with replicated
                    # W_out stationary (all psum rows identical); sigmoid
                    # only row 0
                    a0 = t - tcp
                    hc0, hj0 = divmod(a0, tcs)
                    hsl = h_tiles[hc0][:, hj0 * BL : (hj0 + tcp) * BL]
                    hp = hpsum_pool.tile([H, tcp * BL], F32, tag="hp")
                    hmm_i = nc.tensor.matmul(
                        hp[:], wo_t[:], hsl, start=True, stop=True
                    )
                    add_dep_helper(hmm_i.ins, mm_b_i.ins, False, "fill-idle")
                    hb = hd_pool.tile([1, tcp * BL], F32, tag="hb")
                    hsig_i = nc.scalar.activation(
                        hb[:], hp[0:1, :], AF.Sigmoid, bias=bo_t[0:1, 0:1]
                    )
                    add_dep_helper(hsig_i.ins, tanh_i.ins, False, "fill-idle")
                    nc.sync.dma_start(
                        hd[0:1, a0 * BL : (a0 + tcp) * BL], hb[:]
                    )
            if j == tcs - 1:
                nc.sync.dma_start(
                    hs_r[:, c * tcs * BL : (c + 1) * tcs * BL], cur[:, :]
                )

        nc.sync.dma_start(hs_r[:, t_steps * BL : (t_steps + 1) * BL], stub[:, :])
        # head for the last tcp slice
        la0 = t_steps - tcp
        lc0, lj0 = divmod(la0, tcs)
        lsl = h_tiles[lc0][:, lj0 * BL : (lj0 + tcp) * BL]
        lhp = hpsum_pool.tile([H, tcp * BL], F32, tag="hp")
        nc.tensor.matmul(lhp[:], wo_t[:], lsl, start=True, stop=True)
        lhb = hd_pool.tile([1, tcp * BL], F32, tag="hb")
        nc.scalar.activation(
            lhb[:], lhp[0:1, :], AF.Sigmoid, bias=bo_t[0:1, 0:1]
        )
        nc.sync.dma_start(hd[0:1, la0 * BL : (la0 + tcp) * BL], lhb[:])
        # head for the final h^{(T)}
        hps = hpsum_pool.tile([H, BL], F32, tag="hps")
        nc.tensor.matmul(hps[:], wo_t[:], stub[:, :], start=True, stop=True)
        hbs = hd_pool.tile([1, BL], F32, tag="hbs")
        nc.scalar.activation(
            hbs[:], hps[0:1, :], AF.Sigmoid, bias=bo_t[0:1, 0:1]
        )
        nc.sync.dma_start(
            hd[0:1, t_steps * BL : (t_steps + 1) * BL], hbs[:]
        )

    nc.finalize()
    return nc


def make_in_maps(x, h_0, c_0, W_ih, W_hh, b_ih, b_hh, W_out, b_out):
    x = np.asarray(x, np.float32)
    t_steps = x.shape[1]
    nch = (t_steps + TC - 1) // TC
    Wh = np.asarray(W_hh, np.float32).copy()  # [4H, H]
    Wx = np.concatenate(
        [
            np.asarray(W_ih, np.float32),
            (np.asarray(b_ih, np.float32) + np.asarray(b_hh, np.float32))[:, None],
        ],
        axis=1,
    ).copy()  # [4H, 33]
    Wh[2 * H : 3 * H, :] *= 2.0  # g rows: sigmoid(2g) trick
    Wx[2 * H : 3 * H, :] *= 2.0
    wha = np.ascontiguousarray(Wh[0 : 2 * H, :].T)  # [64, 128] (i, f)
    whb = np.ascontiguousarray(Wh[2 * H : 4 * H, :].T)  # [64, 128] (2g, o)
    wxa = np.ascontiguousarray(Wx[0 : 2 * H, :].T)  # [33, 128]
    wxb = np.ascontiguousarray(Wx[2 * H : 4 * H, :].T)  # [33, 128]
    bf = np.float16
    wo = np.ascontiguousarray(
        np.tile(np.asarray(W_out, np.float32).T, (1, H))
    ).astype(bf)  # [H, H]: column m = W_out for every m
    bscalar = float(np.asarray(b_out, np.float32).reshape(-1)[0])

    xa_full = np.concatenate(
        [x, np.ones((x.shape[0], t_steps, 1), np.float32)], axis=2
    )  # [B, T, 33]
    xa_full = np.ascontiguousarray(xa_full.transpose(2, 1, 0))  # [33, T, B]
    h0_full = np.ascontiguousarray(np.asarray(h_0, np.float32)[0].T)  # [H, B]
    c0_full = np.ascontiguousarray(np.asarray(c_0, np.float32)[0].T) * 0.5

    in_maps = []
    for k in range(NCORES):
        sl = slice(k * BL, (k + 1) * BL)
        in_maps.append(
            {
                "xa": np.ascontiguousarray(xa_full[:, :, sl]).astype(bf),
                "h0": np.ascontiguousarray(h0_full[:, sl]).astype(bf),
                "c0h": np.ascontiguousarray(c0_full[:, sl]).astype(bf),
                "wha": wha.astype(bf),
                "whb": whb.astype(bf),
                "wxa": wxa.astype(bf),
                "wxb": wxb.astype(bf),
                "wo": wo,
                "bo": np.full((1, 1), bscalar, np.float32),
            }
        )
    return in_maps


def assemble_outputs(results, t_steps: int = T):
    bsz = NCORES * BL
    tcs = min(TC, t_steps)
    nch = (t_steps + tcs - 1) // tcs
    hs_out = np.empty((bsz, t_steps, H), np.float32)
    out = np.empty((bsz, t_steps, 1), np.float32)
    for k in range(NCORES):
        hs_k = np.asarray(results[k]["hs"]).astype(np.float32)  # [H, T+1, BL]
        hs_out[k * BL : (k + 1) * BL] = hs_k[:, 1:, :].transpose(2, 1, 0)
        hd_k = np.asarray(results[k]["hd"])  # [1, (T+1)*BL]
        heads = hd_k.reshape(t_steps + 1, BL)
        out[k * BL : (k + 1) * BL, :, 0] = heads[1 : t_steps + 1, :].T
    return out, hs_out


def kernel(x, h_0, c_0, W_ih, W_hh, b_ih, b_hh, W_out, b_out):
    import time

    in_maps = make_in_maps(x, h_0, c_0, W_ih, W_hh, b_ih, b_hh, W_out, b_out)
    nc = build_program(T)
    last_err = None
    for attempt in range(3):
        try:
            res = run_bass_kernel_spmd(nc, in_maps, list(range(NCORES))).results
            return assemble_outputs(res, T)
        except Exception as e:  # transient NRT device errors: retry
            last_err = e
            time.sleep(10.0)
    raise last_err


if __name__ == "__main__":
    nc = build_program(T)
    print("build ok")
